# revision 21
# baseline (speedup 1.0000x reference)
"""BatchAllTripletLoss on 8 Trainium2 NeuronCores.

Strategy
-------
loss = sum_{i,j,k} relu(d(i,j) - d(i,k) + m) * mask / (count + eps) with
d = cosine distance.  Since d_ij - d_ik = S_ik - S_ij (similarities), the
triplet value is t = m - S_ij + S_ik.  For this regime the margin m is much
larger than the cosine similarity spread, so every valid triplet is strictly
positive (verified on device by a per-anchor guard, with an exact O(B^3)
fallback kernel if it ever fails).  Then

  count = sum_i n_pos(i) * n_neg(i)                       (host, exact)
  sum   = sum_i [ n_pos*rs_neg - n_neg*rs_pos + m*n_pos*n_neg ]
  rs_pos(i) = sum_{j in pos(i)} S_ij,  rs_neg(i) = sum_{k in neg(i)} S_ik

Sharding: embeddings are normalized on host (cheap O(B*D)), batch sorted by
label.  4 anchor groups x 2 column halves -> 8 cores, every on-device tile
is [128, 256+] (full partition utilization).  Per core:

  PE:  ps = Xan_g^T-chunks @ [XnT_half | t_half]  (6 matmuls, bf16->f32)
       column 256 of ps = rs_all(i) = sum_{j in half} S_ij  for free
  DVE: ttr(P0 = ps*posmask, reduce add)  -> rs_pos
       ttr(N0 = ps*negmask, reduce min)  -> minN guard
       reduce max P0                     -> maxP guard (0-clamped, safe)
  out: [128,4] partials + rs_all column; host combines halves, computes
       rs_neg = rs_all - S_ii - rs_pos, checks the guard, and divides.
"""

import numpy as np

B, D, NCORES = 512, 768, 8
NG = 4  # anchor groups (128 anchors each)
NH = 2  # column halves (256 columns each)
MA = 64  # anchors per core for the fallback scan kernel
GA = 128  # anchors per group
HC = 256  # columns per half
M2 = 264  # moving free dim: 256 data cols + 1 rowsum col + 7 pad
NCH = D // 128
MARGIN = 0.5
EPS = 1e-8
BIG = 1e9
GUARD_SLACK = 0.05

_PROG_CACHE: dict = {}


def _build_program_fast():
    from contextlib import ExitStack

    import concourse.bacc as bacc
    import concourse.mybir as mybir
    import concourse.tile as tile

    f32 = mybir.dt.float32
    bf16 = mybir.dt.bfloat16
    f8 = mybir.dt.float8e4
    Alu = mybir.AluOpType
    X = mybir.AxisListType.X

    nc = bacc.Bacc("TRN2", target_bir_lowering=False, debug=False, num_devices=NCORES)

    Act = mybir.ActivationFunctionType

    # all inputs partition-major ([128, ...] with the SBUF free layout) so
    # every DMA descriptor row is >=1KB contiguous; x-data in fp8_e4m3
    # (cosine sims only feed a heavily-averaged sum: ~1e-5 final impact)
    xanT = nc.dram_tensor("xanT", [128, NCH * GA], f8, kind="ExternalInput").ap()
    mv = nc.dram_tensor("mv", [128, NCH * M2], f8, kind="ExternalInput").ap()
    msk = nc.dram_tensor("msk", [GA, HC], bf16, kind="ExternalInput").ap()
    o4 = nc.dram_tensor("o4", [GA, 4], f32, kind="ExternalOutput").ap()

    with tile.TileContext(nc) as tc, ExitStack() as ctx:
        pool = ctx.enter_context(tc.tile_pool(name="sb", bufs=1))
        pp = ctx.enter_context(tc.tile_pool(name="ps", bufs=1, space="PSUM"))

        # loads: moving tensor serialized on one queue (chunk pairs complete
        # in order so the matmul pipeline starts early); weights + masks on
        # other queues in parallel
        xanTv = xanT.rearrange("p (c j) -> p c j", c=NCH)
        xanT_t = pool.tile([128, NCH, GA], f8)
        nc.sync.dma_start(xanT_t[:], xanTv)
        mvv = mv.rearrange("p (c j) -> p c j", c=NCH)
        mv_t = pool.tile([128, NCH, M2], f8)
        nc.scalar.dma_start(mv_t[:, 0:3, :], mvv[:, 0:3, :])
        nc.gpsimd.dma_start(mv_t[:, 3:6, :], mvv[:, 3:6, :])
        msk_t = pool.tile([GA, HC], bf16)
        nc.sync.dma_start(msk_t[:], msk)

        # fp8 DoubleRow: each matmul consumes two 128-deep k-chunks
        ps = pp.tile([GA, M2], f32)
        for q in range(0, NCH, 2):
            nc.tensor.matmul(
                ps[:], xanT_t[:, q : q + 2, :], mv_t[:, q : q + 2, :],
                start=(q == 0), stop=(q == NCH - 2),
                perf_mode=mybir.MatmulPerfMode.DoubleRow,
            )

        # OUT4 columns: rs_pos, max(P0) (0-clamped maxpos), min over ALL
        # columns (<= min_neg, conservative guard), rs_all
        OUT4 = pool.tile([GA, 4], f32)
        P0 = pool.tile([GA, HC], bf16)
        S = ps[:, 0:HC]
        nc.vector.tensor_tensor(P0[:], S, msk_t[:], Alu.mult)
        scr = pool.tile([GA, HC], bf16)
        nc.scalar.activation(scr[:], P0[:], Act.Copy, accum_out=OUT4[:, 0:1])
        nc.vector.tensor_reduce(OUT4[:, 2:3], S, X, Alu.min)
        nc.vector.tensor_reduce(OUT4[:, 1:2], P0[:], X, Alu.max)
        nc.vector.tensor_copy(OUT4[:, 3:4], ps[:, HC : HC + 1])
        nc.sync.dma_start(o4, OUT4[:])

    nc.compile()
    return nc


def _fast_in_maps(emb_sorted: np.ndarray):
    """Per-core inputs for the fast program + host-side reference data."""
    import ml_dtypes

    f8 = ml_dtypes.float8_e4m3
    xn = emb_sorted / np.maximum(
        np.linalg.norm(emb_sorted, axis=1, keepdims=True), EPS
    )
    xb = xn.astype(f8)
    sii = (xb.astype(np.float32) ** 2).sum(axis=1)  # S_ii as the device sees it

    mvs = []
    for h in range(NH):
        rows = xb[HC * h : HC * (h + 1)]
        m = np.zeros((D, M2), dtype=f8)
        m[:, 0:HC] = rows.T
        m[:, HC] = rows.astype(np.float32).sum(axis=0).astype(f8)
        # partition-major: [p, q*M2 + j] = m[128q + p, j]
        mpm = m.reshape(NCH, 128, M2).transpose(1, 0, 2).reshape(128, NCH * M2)
        mvs.append(np.ascontiguousarray(mpm))
    xanTs = []
    for g in range(NG):
        a = xb[GA * g : GA * (g + 1)].T  # [D, GA]
        apm = a.reshape(NCH, 128, GA).transpose(1, 0, 2).reshape(128, NCH * GA)
        xanTs.append(np.ascontiguousarray(apm))
    return mvs, xanTs, sii


def _make_masks(lab_sorted: np.ndarray):
    eq = lab_sorted[:, None] == lab_sorted[None, :]
    return eq & ~np.eye(B, dtype=bool)


# ---------------------------------------------------------------------------
# exact O(B^3) fallback (from the always-correct masked-scan formulation)
# ---------------------------------------------------------------------------


class Plan:
    pass


def _make_plan(labels: np.ndarray) -> Plan:
    p = Plan()
    order = np.argsort(labels, kind="stable")
    lab = labels[order]
    nclass = int(lab.max()) + 1
    counts = np.bincount(lab, minlength=nclass).astype(int)
    n = [int(c) for c in counts if c > 0]
    starts = np.concatenate([[0], np.cumsum(n)]).astype(int)
    cls_of = np.searchsorted(starts, np.arange(B), side="right") - 1

    Kpos = max(n)
    Kpos2 = Kpos + (Kpos % 2)
    J2 = Kpos2 // 2

    posmask = np.zeros((NCORES, MA, Kpos2), dtype=np.int8)
    negmask = np.zeros((NCORES, MA, B), dtype=np.int8)
    pm7 = np.zeros((NCORES, len(n), MA, Kpos2), dtype=np.int8)
    for c in range(NCORES):
        for r in range(MA):
            a = MA * c + r
            i = cls_of[a]
            s, nk = starts[i], n[i]
            posmask[c, r, :nk] = 1
            posmask[c, r, a - s] = 0  # j == i
            negmask[c, r, :] = 1
            negmask[c, r, s : s + nk] = 0
            pm7[c, i, r, :] = posmask[c, r, :]

    p.order = order
    p.n = n
    p.starts = starts
    p.Kpos2 = Kpos2
    p.J2 = J2
    p.pm7 = pm7
    p.negmask = negmask
    p.key = tuple(n)
    return p


def _build_program_scan(p: Plan):
    from contextlib import ExitStack

    import concourse.bacc as bacc
    import concourse.mybir as mybir
    import concourse.tile as tile

    f32 = mybir.dt.float32
    bf16 = mybir.dt.bfloat16
    i8 = mybir.dt.int8
    Alu = mybir.AluOpType
    Act = mybir.ActivationFunctionType

    J2, Kpos2 = p.J2, p.Kpos2
    NCLS = len(p.n)

    nc = bacc.Bacc("TRN2", target_bir_lowering=False, debug=False, num_devices=NCORES)

    xT = nc.dram_tensor("xT", [D, B], bf16, kind="ExternalInput").ap()
    xaT = nc.dram_tensor("xaT", [D, MA], bf16, kind="ExternalInput").ap()
    xa = nc.dram_tensor("xa", [MA, D], bf16, kind="ExternalInput").ap()
    pm7 = nc.dram_tensor("pm7", [NCLS, MA, Kpos2], i8, kind="ExternalInput").ap()
    nm = nc.dram_tensor("nm", [MA, B], i8, kind="ExternalInput").ap()
    out = nc.dram_tensor("out", [1, 2], f32, kind="ExternalOutput").ap()

    with tile.TileContext(nc) as tc, ExitStack() as ctx:
        pool = ctx.enter_context(tc.tile_pool(name="sb", bufs=1))
        sqpool = ctx.enter_context(tc.tile_pool(name="sq", bufs=3))
        scrA = ctx.enter_context(tc.tile_pool(name="scrA", bufs=4))
        scrV = ctx.enter_context(tc.tile_pool(name="scrV", bufs=4))
        pp = ctx.enter_context(tc.tile_pool(name="ps", bufs=1, space="PSUM"))

        ones_bf = pool.tile([128, 1], bf16)
        nc.gpsimd.memset(ones_bf[:], 1.0)
        ones_row = pool.tile([1, MA], f32)
        nc.gpsimd.memset(ones_row[:], 1.0)

        xTv = xT.rearrange("(c p) j -> p c j", p=128)
        xT_t = pool.tile([128, NCH, B], bf16)
        for q in range(NCH):
            nc.sync.dma_start(xT_t[:, q, :], xTv[:, q, :])
        xaTv = xaT.rearrange("(c p) j -> p c j", p=128)
        xaT_t = pool.tile([128, NCH, MA], bf16)
        nc.sync.dma_start(xaT_t[:], xaTv)
        xa_t = pool.tile([MA, D], bf16)
        nc.sync.dma_start(xa_t[:], xa)
        pm7_t = pool.tile([MA, NCLS, Kpos2], i8)
        nc.sync.dma_start(pm7_t[:], pm7.rearrange("k m q -> m k q"))
        nm_t = pool.tile([MA, B], i8)
        nc.sync.dma_start(nm_t[:], nm)

        # column norms ssq[j] = sum_d x[d,j]^2
        ps_ssq = pp.tile([1, B], f32)
        for q in range(NCH):
            sq = sqpool.tile([128, B], bf16, tag="sq")
            nc.scalar.activation(sq[:], xT_t[:, q, :], Act.Square)
            nc.tensor.matmul(
                ps_ssq[:], ones_bf[:], sq[:], start=(q == 0), stop=(q == NCH - 1)
            )
        nrm = pool.tile([1, B], f32)
        nc.scalar.activation(nrm[:], ps_ssq[:], Act.Sqrt)
        invn = pool.tile([1, B], f32)
        nc.vector.reciprocal(invn[:], nrm[:])

        # anchor norms
        scr_a = pool.tile([MA, D], bf16)
        ssqa = pool.tile([MA, 1], f32)
        nc.scalar.activation(scr_a[:], xa_t[:], Act.Square, accum_out=ssqa[:])
        nrma = pool.tile([MA, 1], f32)
        nc.scalar.activation(nrma[:], ssqa[:], Act.Sqrt)
        invna = pool.tile([MA, 1], f32)
        nc.vector.reciprocal(invna[:], nrma[:])

        # t = d_ij - d_ik + m = (m - S_ij) + S_ik
        ps_G = pp.tile([MA, B], f32)
        for q in range(NCH):
            nc.tensor.matmul(
                ps_G[:], xaT_t[:, q, :], xT_t[:, q, :],
                start=(q == 0), stop=(q == NCH - 1),
            )
        ps_B = pp.tile([MA, B], f32)
        nc.tensor.matmul(ps_B[:], ones_row[:], invn[:], start=True, stop=True)
        invnB = pool.tile([MA, B], f32)
        nc.scalar.activation(invnB[:], ps_B[:], Act.Copy)
        Sm = pool.tile([MA, B], bf16)
        nc.vector.scalar_tensor_tensor(
            Sm[:], ps_G[:], invna[:], invnB[:], Alu.mult, Alu.mult
        )
        ms = pool.tile([MA, B], f32)
        nc.vector.tensor_scalar(ms[:], Sm[:], -1.0, MARGIN, Alu.mult, Alu.add)

        # POS bias = m - S_ij (compacted, data-driven classes)
        posf = pool.tile([MA, Kpos2], f32)
        nc.gpsimd.memset(posf[:], -BIG)
        for i in range(NCLS):
            s, nk = p.starts[i], p.n[i]
            nc.vector.copy_predicated(
                posf[:, 0:nk], pm7_t[:, i, 0:nk], ms[:, s : s + nk]
            )
        POSst = pool.tile([128, J2], f32)
        nc.gpsimd.memset(POSst[:], -BIG)
        pe = posf.rearrange("p (a two) -> p two a", two=2)
        nc.vector.tensor_copy(POSst[0:MA, :], pe[:, 0, :])
        nc.sync.dma_start(POSst[64 : 64 + MA, :], pe[:, 1, :])

        # NEG = S_ik (dense bf16; same-class columns -> -BIG)
        NEGS = pool.tile([128, B], bf16)
        nc.gpsimd.memset(NEGS[:], -BIG)
        nc.vector.copy_predicated(NEGS[0:MA, :], nm_t[:], Sm[:])
        nc.sync.dma_start(NEGS[64 : 64 + MA, :], NEGS[0:MA, :])

        POSng = pool.tile([128, J2], f32)
        nc.vector.tensor_scalar_mul(POSng[:], POSst[:], -1.0)

        cnt_acc = pool.tile([128, B], bf16)
        nc.gpsimd.memset(cnt_acc[:], 0.0)
        ps_sum = pp.tile([1, B], f32)
        for jj in range(J2):
            if jj % 7 < 4:
                sA = scrA.tile([128, B], bf16, tag="sA")
                nc.scalar.activation(
                    sA[:], NEGS[:], Act.Relu, bias=POSst[:, jj : jj + 1]
                )
            else:
                sA = scrV.tile([128, B], bf16, tag="sV")
                nc.vector.tensor_scalar(
                    sA[:], NEGS[:], POSst[:, jj : jj + 1], 0.0, Alu.add, Alu.max
                )
            nc.tensor.matmul(
                ps_sum[:], ones_bf[:], sA[:],
                start=(jj == 0), stop=(jj == J2 - 1), skip_group_check=True,
            )
            nc.vector.scalar_tensor_tensor(
                cnt_acc[:], NEGS[:], POSng[:, jj : jj + 1], cnt_acc[:],
                Alu.is_gt, Alu.add,
            )

        ps_cnt = pp.tile([1, B], f32)
        nc.tensor.matmul(ps_cnt[:], ones_bf[:], cnt_acc[:], start=True, stop=True)
        outs = pool.tile([1, 2], f32)
        scr1 = pool.tile([1, B], f32)
        nc.scalar.activation(scr1[:], ps_sum[:], Act.Copy, accum_out=outs[:, 0:1])
        scr2 = pool.tile([1, B], f32)
        nc.scalar.activation(scr2[:], ps_cnt[:], Act.Copy, accum_out=outs[:, 1:2])
        nc.sync.dma_start(out, outs[:])

    nc.compile()
    return nc


def _scan_in_maps(p: Plan, emb: np.ndarray):
    import ml_dtypes

    bf = ml_dtypes.bfloat16
    xs = np.ascontiguousarray(emb[p.order])
    xT = np.ascontiguousarray(xs.T.astype(bf))
    maps = []
    for c in range(NCORES):
        xa = xs[MA * c : MA * (c + 1)]
        maps.append(
            {
                "xT": xT,
                "xaT": np.ascontiguousarray(xa.T.astype(bf)),
                "xa": np.ascontiguousarray(xa.astype(bf)),
                "nm": p.negmask[c],
                "pm7": p.pm7[c],
            }
        )
    return maps


LAST_RESULT = None  # BassKernelResults of the most recent run (for profiling)


def kernel(embeddings, labels):
    global LAST_RESULT
    import os

    from concourse.bass_utils import run_bass_kernel_spmd

    emb = np.ascontiguousarray(np.asarray(embeddings, dtype=np.float32))
    lab = np.asarray(labels).astype(np.int64)
    order = np.argsort(lab, kind="stable")
    lab_s = lab[order]
    emb_s = emb[order]

    trace = bool(int(os.environ.get("TRIPLET_TRACE", "0")))
    kw = {}
    if os.environ.get("TRIPLET_TMPDIR"):
        kw["tmpdir"] = os.environ["TRIPLET_TMPDIR"]

    if "fast" not in _PROG_CACHE:
        _PROG_CACHE["fast"] = _build_program_fast()

    mvs, xanTs, sii = _fast_in_maps(emb_s)
    pm = _make_masks(lab_s)
    import ml_dtypes

    bf = ml_dtypes.bfloat16
    in_maps = []
    for c in range(NCORES):
        g, h = c // NH, c % NH
        m = np.ascontiguousarray(
            pm[GA * g : GA * (g + 1), HC * h : HC * (h + 1)].astype(bf)
        )
        in_maps.append({"xanT": xanTs[g], "mv": mvs[h], "msk": m})

    LAST_RESULT = run_bass_kernel_spmd(
        _PROG_CACHE["fast"], in_maps, list(range(NCORES)), trace=trace, **kw
    )
    res = LAST_RESULT.results

    # per-anchor combine across halves
    rs_pos = np.zeros(B, dtype=np.float64)
    rs_all = np.zeros(B, dtype=np.float64)
    maxP = np.full(B, -np.inf)
    minN = np.full(B, np.inf)
    for c in range(NCORES):
        g = c // NH
        o4 = np.asarray(res[c]["o4"], dtype=np.float64)
        sl = slice(GA * g, GA * (g + 1))
        rs_pos[sl] += o4[:, 0]
        maxP[sl] = np.maximum(maxP[sl], o4[:, 1])
        minN[sl] = np.minimum(minN[sl], o4[:, 2])
        rs_all[sl] += o4[:, 3]

    n_pos = (lab_s[:, None] == lab_s[None, :]).sum(axis=1) - 1
    n_neg = B - n_pos - 1
    n_valid = float((n_pos * n_neg).sum())

    worst = float(np.max(maxP - minN))
    if worst < MARGIN - GUARD_SLACK:
        rs_neg = rs_all - sii.astype(np.float64) - rs_pos
        V = n_pos * rs_neg - n_neg * rs_pos + MARGIN * n_pos * n_neg
        return np.float32(V.sum() / (n_valid + EPS))

    # fallback: full O(B^3) masked scan (always correct)
    p = _make_plan(lab)
    skey = ("scan", p.key)
    if skey not in _PROG_CACHE:
        _PROG_CACHE[skey] = _build_program_scan(p)
    LAST_RESULT = run_bass_kernel_spmd(
        _PROG_CACHE[skey], _scan_in_maps(p, emb), list(range(NCORES)),
        trace=trace, **kw,
    )
    S = 0.0
    C = 0.0
    for r in LAST_RESULT.results:
        o = np.asarray(r["out"], dtype=np.float64).reshape(-1)
        S += o[0]
        C += o[1]
    return np.float32(S / (C + EPS))


# revision 22
# speedup vs baseline: 1.1956x; 1.1956x over previous
"""BatchAllTripletLoss on 8 Trainium2 NeuronCores.

Strategy
-------
loss = sum_{i,j,k} relu(d(i,j) - d(i,k) + m) * mask / (count + eps) with
d = cosine distance.  Since d_ij - d_ik = S_ik - S_ij (similarities), the
triplet value is t = m - S_ij + S_ik.  For this regime the margin m is much
larger than the cosine similarity spread, so every valid triplet is strictly
positive (verified on device by a per-anchor guard, with an exact O(B^3)
fallback kernel if it ever fails).  Then

  count = sum_i n_pos(i) * n_neg(i)                       (host, exact)
  sum   = sum_i [ n_pos*rs_neg - n_neg*rs_pos + m*n_pos*n_neg ]
  rs_pos(i) = sum_{j in pos(i)} S_ij,  rs_neg(i) = sum_{k in neg(i)} S_ik

Sharding: embeddings are normalized on host (cheap O(B*D)), batch sorted by
label.  4 anchor groups x 2 column halves -> 8 cores, every on-device tile
is [128, 256+] (full partition utilization).  Per core:

  PE:  ps = Xan_g^T-chunks @ [XnT_half | t_half]  (6 matmuls, bf16->f32)
       column 256 of ps = rs_all(i) = sum_{j in half} S_ij  for free
  DVE: ttr(P0 = ps*posmask, reduce add)  -> rs_pos
       ttr(N0 = ps*negmask, reduce min)  -> minN guard
       reduce max P0                     -> maxP guard (0-clamped, safe)
  out: [128,4] partials + rs_all column; host combines halves, computes
       rs_neg = rs_all - S_ii - rs_pos, checks the guard, and divides.
"""

import numpy as np

B, D, NCORES = 512, 768, 8
NG = 4  # anchor groups (128 anchors each)
NH = 2  # column halves (256 columns each)
MA = 64  # anchors per core for the fallback scan kernel
GA = 128  # anchors per group
HC = 256  # columns per half
M2 = 264  # moving free dim: 256 data cols + 1 rowsum col + 7 pad
NCH = D // 128
MARGIN = 0.5
EPS = 1e-8
BIG = 1e9
GUARD_SLACK = 0.05

_PROG_CACHE: dict = {}


def _build_program_fast():
    from contextlib import ExitStack

    import concourse.bacc as bacc
    import concourse.mybir as mybir
    import concourse.tile as tile

    f32 = mybir.dt.float32
    bf16 = mybir.dt.bfloat16
    f8 = mybir.dt.float8e4
    Alu = mybir.AluOpType
    X = mybir.AxisListType.X

    nc = bacc.Bacc("TRN2", target_bir_lowering=False, debug=False, num_devices=NCORES)

    Act = mybir.ActivationFunctionType

    # all inputs partition-major ([128, ...] with the SBUF free layout) so
    # every DMA descriptor row is >=1KB contiguous; x-data in fp8_e4m3
    # (cosine sims only feed a heavily-averaged sum: ~1e-5 final impact)
    xanT = nc.dram_tensor("xanT", [128, NCH * GA], f8, kind="ExternalInput").ap()
    mv = nc.dram_tensor("mv", [128, NCH * M2], f8, kind="ExternalInput").ap()
    msk = nc.dram_tensor("msk", [GA, HC], bf16, kind="ExternalInput").ap()
    o4 = nc.dram_tensor("o4", [GA, 4], f32, kind="ExternalOutput").ap()

    with tile.TileContext(nc) as tc, ExitStack() as ctx:
        pool = ctx.enter_context(tc.tile_pool(name="sb", bufs=1))
        pp = ctx.enter_context(tc.tile_pool(name="ps", bufs=1, space="PSUM"))

        # loads: moving tensor serialized on one queue (chunk pairs complete
        # in order so the matmul pipeline starts early); weights + masks on
        # other queues in parallel
        xanTv = xanT.rearrange("p (c j) -> p c j", c=NCH)
        xanT_t = pool.tile([128, NCH, GA], f8)
        nc.sync.dma_start(xanT_t[:], xanTv)
        mvv = mv.rearrange("p (c j) -> p c j", c=NCH)
        mv_t = pool.tile([128, NCH, M2], f8)
        nc.scalar.dma_start(mv_t[:], mvv)
        msk_t = pool.tile([GA, HC], bf16)
        nc.sync.dma_start(msk_t[:], msk)

        # fp8 DoubleRow: each matmul consumes two 128-deep k-chunks
        ps = pp.tile([GA, M2], f32)
        for q in range(0, NCH, 2):
            nc.tensor.matmul(
                ps[:], xanT_t[:, q : q + 2, :], mv_t[:, q : q + 2, :],
                start=(q == 0), stop=(q == NCH - 2),
                perf_mode=mybir.MatmulPerfMode.DoubleRow,
            )

        # OUT4 columns: rs_pos, max(P0) (0-clamped maxpos), min over ALL
        # columns (<= min_neg, conservative guard), rs_all
        OUT4 = pool.tile([GA, 4], f32)
        P0 = pool.tile([GA, HC], bf16)
        S = ps[:, 0:HC]
        nc.vector.tensor_tensor(P0[:], S, msk_t[:], Alu.mult)
        scr = pool.tile([GA, HC], bf16)
        nc.scalar.activation(scr[:], P0[:], Act.Copy, accum_out=OUT4[:, 0:1])
        nc.vector.tensor_reduce(OUT4[:, 2:3], S, X, Alu.min)
        nc.vector.tensor_reduce(OUT4[:, 1:2], P0[:], X, Alu.max)
        nc.vector.tensor_copy(OUT4[:, 3:4], ps[:, HC : HC + 1])
        nc.sync.dma_start(o4, OUT4[:])

    nc.compile()
    return nc


def _fast_in_maps(emb_sorted: np.ndarray):
    """Per-core inputs for the fast program + host-side reference data."""
    import ml_dtypes

    f8 = ml_dtypes.float8_e4m3
    xn = emb_sorted / np.maximum(
        np.linalg.norm(emb_sorted, axis=1, keepdims=True), EPS
    )
    xb = xn.astype(f8)
    sii = (xb.astype(np.float32) ** 2).sum(axis=1)  # S_ii as the device sees it

    mvs = []
    for h in range(NH):
        rows = xb[HC * h : HC * (h + 1)]
        m = np.zeros((D, M2), dtype=f8)
        m[:, 0:HC] = rows.T
        m[:, HC] = rows.astype(np.float32).sum(axis=0).astype(f8)
        # partition-major: [p, q*M2 + j] = m[128q + p, j]
        mpm = m.reshape(NCH, 128, M2).transpose(1, 0, 2).reshape(128, NCH * M2)
        mvs.append(np.ascontiguousarray(mpm))
    xanTs = []
    for g in range(NG):
        a = xb[GA * g : GA * (g + 1)].T  # [D, GA]
        apm = a.reshape(NCH, 128, GA).transpose(1, 0, 2).reshape(128, NCH * GA)
        xanTs.append(np.ascontiguousarray(apm))
    return mvs, xanTs, sii


def _make_masks(lab_sorted: np.ndarray):
    eq = lab_sorted[:, None] == lab_sorted[None, :]
    return eq & ~np.eye(B, dtype=bool)


# ---------------------------------------------------------------------------
# exact O(B^3) fallback (from the always-correct masked-scan formulation)
# ---------------------------------------------------------------------------


class Plan:
    pass


def _make_plan(labels: np.ndarray) -> Plan:
    p = Plan()
    order = np.argsort(labels, kind="stable")
    lab = labels[order]
    nclass = int(lab.max()) + 1
    counts = np.bincount(lab, minlength=nclass).astype(int)
    n = [int(c) for c in counts if c > 0]
    starts = np.concatenate([[0], np.cumsum(n)]).astype(int)
    cls_of = np.searchsorted(starts, np.arange(B), side="right") - 1

    Kpos = max(n)
    Kpos2 = Kpos + (Kpos % 2)
    J2 = Kpos2 // 2

    posmask = np.zeros((NCORES, MA, Kpos2), dtype=np.int8)
    negmask = np.zeros((NCORES, MA, B), dtype=np.int8)
    pm7 = np.zeros((NCORES, len(n), MA, Kpos2), dtype=np.int8)
    for c in range(NCORES):
        for r in range(MA):
            a = MA * c + r
            i = cls_of[a]
            s, nk = starts[i], n[i]
            posmask[c, r, :nk] = 1
            posmask[c, r, a - s] = 0  # j == i
            negmask[c, r, :] = 1
            negmask[c, r, s : s + nk] = 0
            pm7[c, i, r, :] = posmask[c, r, :]

    p.order = order
    p.n = n
    p.starts = starts
    p.Kpos2 = Kpos2
    p.J2 = J2
    p.pm7 = pm7
    p.negmask = negmask
    p.key = tuple(n)
    return p


def _build_program_scan(p: Plan):
    from contextlib import ExitStack

    import concourse.bacc as bacc
    import concourse.mybir as mybir
    import concourse.tile as tile

    f32 = mybir.dt.float32
    bf16 = mybir.dt.bfloat16
    i8 = mybir.dt.int8
    Alu = mybir.AluOpType
    Act = mybir.ActivationFunctionType

    J2, Kpos2 = p.J2, p.Kpos2
    NCLS = len(p.n)

    nc = bacc.Bacc("TRN2", target_bir_lowering=False, debug=False, num_devices=NCORES)

    xT = nc.dram_tensor("xT", [D, B], bf16, kind="ExternalInput").ap()
    xaT = nc.dram_tensor("xaT", [D, MA], bf16, kind="ExternalInput").ap()
    xa = nc.dram_tensor("xa", [MA, D], bf16, kind="ExternalInput").ap()
    pm7 = nc.dram_tensor("pm7", [NCLS, MA, Kpos2], i8, kind="ExternalInput").ap()
    nm = nc.dram_tensor("nm", [MA, B], i8, kind="ExternalInput").ap()
    out = nc.dram_tensor("out", [1, 2], f32, kind="ExternalOutput").ap()

    with tile.TileContext(nc) as tc, ExitStack() as ctx:
        pool = ctx.enter_context(tc.tile_pool(name="sb", bufs=1))
        sqpool = ctx.enter_context(tc.tile_pool(name="sq", bufs=3))
        scrA = ctx.enter_context(tc.tile_pool(name="scrA", bufs=4))
        scrV = ctx.enter_context(tc.tile_pool(name="scrV", bufs=4))
        pp = ctx.enter_context(tc.tile_pool(name="ps", bufs=1, space="PSUM"))

        ones_bf = pool.tile([128, 1], bf16)
        nc.gpsimd.memset(ones_bf[:], 1.0)
        ones_row = pool.tile([1, MA], f32)
        nc.gpsimd.memset(ones_row[:], 1.0)

        xTv = xT.rearrange("(c p) j -> p c j", p=128)
        xT_t = pool.tile([128, NCH, B], bf16)
        for q in range(NCH):
            nc.sync.dma_start(xT_t[:, q, :], xTv[:, q, :])
        xaTv = xaT.rearrange("(c p) j -> p c j", p=128)
        xaT_t = pool.tile([128, NCH, MA], bf16)
        nc.sync.dma_start(xaT_t[:], xaTv)
        xa_t = pool.tile([MA, D], bf16)
        nc.sync.dma_start(xa_t[:], xa)
        pm7_t = pool.tile([MA, NCLS, Kpos2], i8)
        nc.sync.dma_start(pm7_t[:], pm7.rearrange("k m q -> m k q"))
        nm_t = pool.tile([MA, B], i8)
        nc.sync.dma_start(nm_t[:], nm)

        # column norms ssq[j] = sum_d x[d,j]^2
        ps_ssq = pp.tile([1, B], f32)
        for q in range(NCH):
            sq = sqpool.tile([128, B], bf16, tag="sq")
            nc.scalar.activation(sq[:], xT_t[:, q, :], Act.Square)
            nc.tensor.matmul(
                ps_ssq[:], ones_bf[:], sq[:], start=(q == 0), stop=(q == NCH - 1)
            )
        nrm = pool.tile([1, B], f32)
        nc.scalar.activation(nrm[:], ps_ssq[:], Act.Sqrt)
        invn = pool.tile([1, B], f32)
        nc.vector.reciprocal(invn[:], nrm[:])

        # anchor norms
        scr_a = pool.tile([MA, D], bf16)
        ssqa = pool.tile([MA, 1], f32)
        nc.scalar.activation(scr_a[:], xa_t[:], Act.Square, accum_out=ssqa[:])
        nrma = pool.tile([MA, 1], f32)
        nc.scalar.activation(nrma[:], ssqa[:], Act.Sqrt)
        invna = pool.tile([MA, 1], f32)
        nc.vector.reciprocal(invna[:], nrma[:])

        # t = d_ij - d_ik + m = (m - S_ij) + S_ik
        ps_G = pp.tile([MA, B], f32)
        for q in range(NCH):
            nc.tensor.matmul(
                ps_G[:], xaT_t[:, q, :], xT_t[:, q, :],
                start=(q == 0), stop=(q == NCH - 1),
            )
        ps_B = pp.tile([MA, B], f32)
        nc.tensor.matmul(ps_B[:], ones_row[:], invn[:], start=True, stop=True)
        invnB = pool.tile([MA, B], f32)
        nc.scalar.activation(invnB[:], ps_B[:], Act.Copy)
        Sm = pool.tile([MA, B], bf16)
        nc.vector.scalar_tensor_tensor(
            Sm[:], ps_G[:], invna[:], invnB[:], Alu.mult, Alu.mult
        )
        ms = pool.tile([MA, B], f32)
        nc.vector.tensor_scalar(ms[:], Sm[:], -1.0, MARGIN, Alu.mult, Alu.add)

        # POS bias = m - S_ij (compacted, data-driven classes)
        posf = pool.tile([MA, Kpos2], f32)
        nc.gpsimd.memset(posf[:], -BIG)
        for i in range(NCLS):
            s, nk = p.starts[i], p.n[i]
            nc.vector.copy_predicated(
                posf[:, 0:nk], pm7_t[:, i, 0:nk], ms[:, s : s + nk]
            )
        POSst = pool.tile([128, J2], f32)
        nc.gpsimd.memset(POSst[:], -BIG)
        pe = posf.rearrange("p (a two) -> p two a", two=2)
        nc.vector.tensor_copy(POSst[0:MA, :], pe[:, 0, :])
        nc.sync.dma_start(POSst[64 : 64 + MA, :], pe[:, 1, :])

        # NEG = S_ik (dense bf16; same-class columns -> -BIG)
        NEGS = pool.tile([128, B], bf16)
        nc.gpsimd.memset(NEGS[:], -BIG)
        nc.vector.copy_predicated(NEGS[0:MA, :], nm_t[:], Sm[:])
        nc.sync.dma_start(NEGS[64 : 64 + MA, :], NEGS[0:MA, :])

        POSng = pool.tile([128, J2], f32)
        nc.vector.tensor_scalar_mul(POSng[:], POSst[:], -1.0)

        cnt_acc = pool.tile([128, B], bf16)
        nc.gpsimd.memset(cnt_acc[:], 0.0)
        ps_sum = pp.tile([1, B], f32)
        for jj in range(J2):
            if jj % 7 < 4:
                sA = scrA.tile([128, B], bf16, tag="sA")
                nc.scalar.activation(
                    sA[:], NEGS[:], Act.Relu, bias=POSst[:, jj : jj + 1]
                )
            else:
                sA = scrV.tile([128, B], bf16, tag="sV")
                nc.vector.tensor_scalar(
                    sA[:], NEGS[:], POSst[:, jj : jj + 1], 0.0, Alu.add, Alu.max
                )
            nc.tensor.matmul(
                ps_sum[:], ones_bf[:], sA[:],
                start=(jj == 0), stop=(jj == J2 - 1), skip_group_check=True,
            )
            nc.vector.scalar_tensor_tensor(
                cnt_acc[:], NEGS[:], POSng[:, jj : jj + 1], cnt_acc[:],
                Alu.is_gt, Alu.add,
            )

        ps_cnt = pp.tile([1, B], f32)
        nc.tensor.matmul(ps_cnt[:], ones_bf[:], cnt_acc[:], start=True, stop=True)
        outs = pool.tile([1, 2], f32)
        scr1 = pool.tile([1, B], f32)
        nc.scalar.activation(scr1[:], ps_sum[:], Act.Copy, accum_out=outs[:, 0:1])
        scr2 = pool.tile([1, B], f32)
        nc.scalar.activation(scr2[:], ps_cnt[:], Act.Copy, accum_out=outs[:, 1:2])
        nc.sync.dma_start(out, outs[:])

    nc.compile()
    return nc


def _scan_in_maps(p: Plan, emb: np.ndarray):
    import ml_dtypes

    bf = ml_dtypes.bfloat16
    xs = np.ascontiguousarray(emb[p.order])
    xT = np.ascontiguousarray(xs.T.astype(bf))
    maps = []
    for c in range(NCORES):
        xa = xs[MA * c : MA * (c + 1)]
        maps.append(
            {
                "xT": xT,
                "xaT": np.ascontiguousarray(xa.T.astype(bf)),
                "xa": np.ascontiguousarray(xa.astype(bf)),
                "nm": p.negmask[c],
                "pm7": p.pm7[c],
            }
        )
    return maps


LAST_RESULT = None  # BassKernelResults of the most recent run (for profiling)


def kernel(embeddings, labels):
    global LAST_RESULT
    import os

    from concourse.bass_utils import run_bass_kernel_spmd

    emb = np.ascontiguousarray(np.asarray(embeddings, dtype=np.float32))
    lab = np.asarray(labels).astype(np.int64)
    order = np.argsort(lab, kind="stable")
    lab_s = lab[order]
    emb_s = emb[order]

    trace = bool(int(os.environ.get("TRIPLET_TRACE", "0")))
    kw = {}
    if os.environ.get("TRIPLET_TMPDIR"):
        kw["tmpdir"] = os.environ["TRIPLET_TMPDIR"]

    if "fast" not in _PROG_CACHE:
        _PROG_CACHE["fast"] = _build_program_fast()

    mvs, xanTs, sii = _fast_in_maps(emb_s)
    pm = _make_masks(lab_s)
    import ml_dtypes

    bf = ml_dtypes.bfloat16
    in_maps = []
    for c in range(NCORES):
        g, h = c // NH, c % NH
        m = np.ascontiguousarray(
            pm[GA * g : GA * (g + 1), HC * h : HC * (h + 1)].astype(bf)
        )
        in_maps.append({"xanT": xanTs[g], "mv": mvs[h], "msk": m})

    LAST_RESULT = run_bass_kernel_spmd(
        _PROG_CACHE["fast"], in_maps, list(range(NCORES)), trace=trace, **kw
    )
    res = LAST_RESULT.results

    # per-anchor combine across halves
    rs_pos = np.zeros(B, dtype=np.float64)
    rs_all = np.zeros(B, dtype=np.float64)
    maxP = np.full(B, -np.inf)
    minN = np.full(B, np.inf)
    for c in range(NCORES):
        g = c // NH
        o4 = np.asarray(res[c]["o4"], dtype=np.float64)
        sl = slice(GA * g, GA * (g + 1))
        rs_pos[sl] += o4[:, 0]
        maxP[sl] = np.maximum(maxP[sl], o4[:, 1])
        minN[sl] = np.minimum(minN[sl], o4[:, 2])
        rs_all[sl] += o4[:, 3]

    n_pos = (lab_s[:, None] == lab_s[None, :]).sum(axis=1) - 1
    n_neg = B - n_pos - 1
    n_valid = float((n_pos * n_neg).sum())

    worst = float(np.max(maxP - minN))
    if worst < MARGIN - GUARD_SLACK:
        rs_neg = rs_all - sii.astype(np.float64) - rs_pos
        V = n_pos * rs_neg - n_neg * rs_pos + MARGIN * n_pos * n_neg
        return np.float32(V.sum() / (n_valid + EPS))

    # fallback: full O(B^3) masked scan (always correct)
    p = _make_plan(lab)
    skey = ("scan", p.key)
    if skey not in _PROG_CACHE:
        _PROG_CACHE[skey] = _build_program_scan(p)
    LAST_RESULT = run_bass_kernel_spmd(
        _PROG_CACHE[skey], _scan_in_maps(p, emb), list(range(NCORES)),
        trace=trace, **kw,
    )
    S = 0.0
    C = 0.0
    for r in LAST_RESULT.results:
        o = np.asarray(r["out"], dtype=np.float64).reshape(-1)
        S += o[0]
        C += o[1]
    return np.float32(S / (C + EPS))
